# revision 96
# baseline (speedup 1.0000x reference)
"""Trainium2 Bass kernel for ternary-quantized attention (BitNet-style).

Host contract: kernel(x, w_qkv) -> [16,1025,768] fp32.
Shards B=16 over 8 cores (2 batches/core), replicates the ternary weight.

Math (matches fp32 reference to ~0.8% rel err):
  - w ternarized on host to {-1,0,1}; s_w folded out (scale-invariant l1norm).
  - qkv = x_hi @ wt + x_lo8 @ wt8: hi pass in bf16 (1 cyc/row), lo residual
    as e4m3(x_lo*512) via fp8 DoubleRow (0.5 cyc/row) with 1/512 folded into
    wt8 = wq/512 (exact e4m3 subnormal), both accumulating into one PSUM
    group. Host does the split; n=1024 tail row qkv computed on host in fp32.
    Total qkv noise ~2^-14 -> rel err 0.0165 (< 2e-2 gate).
  - q/k/v quantize: u = t / (l1_row * s_const), s_const = 1/64 + 1e-5;
    ternary = sign(bf16(u + 192) - 192)   [bf16 write rounds to int, RNE]
  - attn_int = q_q @ k_q^T (exact ternary bf16 matmuls, fp32 accum)
  - per-(b,h) scale: t = mean|attn_int| + EPS/(scale*s_const^2); rho = 1/t
  - y = clamp(bf16(attn_int*rho + 192), 191, 193) in {191,192,193}
  - out = (y @ v_q - 192*colsum(v_q)) * (scale * s_const^3 * t)
    [-192*colsum folded in as a rank-1 correction matmul into PSUM]
  - the m=1024 attn row is computed for all 12 heads at once via a
    block-diagonal k-tail stationary matrix (per batch, not per head).
"""
import sys, os
sys.path.insert(0, "/opt/trn_rl_repo")
import numpy as np
import ml_dtypes
from contextlib import ExitStack

import concourse.bass as bass
import concourse.tile as tile
from concourse import bacc
from concourse import mybir
from concourse import bass_isa
from concourse.bass_utils import run_bass_kernel_spmd

EPS = 1e-5
B, N, C, H, D = 16, 1025, 768, 12, 64
BPC = B // 8  # batches per core
SCALE = float(D) ** -0.5
S_CONST = np.float32(1.0 / D) + np.float32(EPS)
C_EPS = np.float32(EPS) / (np.float32(SCALE) * S_CONST * S_CONST)
KAPPA = np.float32(SCALE) * S_CONST * S_CONST * S_CONST
M192 = 192.0

F32 = mybir.dt.float32
F32R = mybir.dt.float32r
BF16 = mybir.dt.bfloat16
F8E4 = mybir.dt.float8e4

N_CHUNKS = [(i * 128, 128) for i in range(8)] + [(1024, 1)]
M_FULL = [(i * 128, 128) for i in range(8)]  # m-tail row handled batched
AS = 1026  # attn_sb per-m-chunk column stride (even)
QKV_TILES = [(0, 512), (512, 512), (1024, 512), (1536, 512), (2048, 256)]
NT2 = ((0, 512), (512, 512))


def build_nc():
    nc = bacc.Bacc("TRN2", target_bir_lowering=False, debug=False,
                   enable_asserts=False, num_devices=8)
    for val in (-M192, M192):
        t = nc.alloc_sbuf_tensor(f"const-f32-{val}", [128, 1], F32)
        nc.gpsimd.memset(t.ap(), val)
        nc.const_aps.aps[(F32, val)] = t.ap()
    nc.all_engine_barrier()
    xh_d = nc.dram_tensor("x_hi", [BPC, C, N], BF16, kind="ExternalInput").ap()
    # x_lo as e4m3(x_lo*512), c-dim grouped (3 pairs, 2 subtiles, 128) for DoubleRow
    xl_d = nc.dram_tensor("x_lo8", [BPC, 3, 128, 2 * 1040], F8E4, kind="ExternalInput").ap()
    w8_d = nc.dram_tensor("wt8", [3, 128, 2 * 3 * C], F8E4, kind="ExternalInput").ap()
    wt_d = nc.dram_tensor("wt_bf", [C, 3 * C], BF16, kind="ExternalInput").ap()
    qt_d = nc.dram_tensor("qkvt", [BPC, 3 * C], F32, kind="ExternalInput").ap()
    id_d = nc.dram_tensor("ident", [128, 128], BF16, kind="ExternalInput").ap()
    on_d = nc.dram_tensor("ones128", [128, 1], F32, kind="ExternalInput").ap()
    y_d = nc.dram_tensor("y_sh", [BPC, C, N], F32, kind="ExternalOutput").ap()

    with tile.TileContext(nc) as tc, ExitStack() as ctx:
        const_p = ctx.enter_context(tc.tile_pool(name="consts", bufs=1))
        qt_p = ctx.enter_context(tc.tile_pool(name="qt", bufs=6 * BPC))
        kt_p = ctx.enter_context(tc.tile_pool(name="kt", bufs=6 * BPC))
        vq_p = ctx.enter_context(tc.tile_pool(name="vq", bufs=BPC))
        cv_p = ctx.enter_context(tc.tile_pool(name="cv", bufs=BPC))

        ident = const_p.tile([128, 128], BF16, tag="ident")
        nc.sync.dma_start(ident[:], id_d)
        ones128 = const_p.tile([128, 1], F32, tag="ones")
        nc.sync.dma_start(ones128[:], on_d)
        ones128b = const_p.tile([128, 1], BF16, tag="onesb")
        nc.vector.tensor_copy(ones128b[:], ones128[:])
        identf = const_p.tile([128, 128], F32, tag="identf")
        nc.vector.tensor_copy(identf[:], ident[:])

        qT = [[qt_p.tile([128, N], BF16, tag="qt", name=f"qT_{b}_{j}") for j in range(6)] for b in range(BPC)]
        kT = [[kt_p.tile([128, N], BF16, tag="kt", name=f"kT_{b}_{j}") for j in range(6)] for b in range(BPC)]
        vq = [vq_p.tile([128, 9 * C], BF16, tag="vq", name=f"vq_{b}") for b in range(BPC)]
        # colsum(vq) as columns: [128, 6]; col j = c-chunk j (2 heads stacked)
        cvcol = [cv_p.tile([128, 8], F32, tag="cv", name=f"cv_{b}") for b in range(BPC)]

        _PH = os.environ.get("KERNEL_PHASE", "full")
        # ================= PHASE A: qkv + quantize + transpose =================
        with tc.tile_pool(name="wt", bufs=6) as wt_p, \
             tc.tile_pool(name="xs", bufs=6) as xs_p, \
             tc.tile_pool(name="qkvsb", bufs=2) as qkvsb_p, \
             tc.tile_pool(name="small_a", bufs=4) as small_p, \
             tc.tile_pool(name="y192", bufs=2) as y192_p, \
             tc.tile_pool(name="qkq", bufs=2) as qkq_p, \
             tc.tile_pool(name="ps_qkv", bufs=5, space="PSUM") as ps_qkv, \
             tc.tile_pool(name="ps_tr", bufs=3, space="PSUM") as ps_tr:
            wt = []
            for c in range(6):
                t = wt_p.tile([128, 3 * C], BF16, tag="wt")
                nc.sync.dma_start(t[:], wt_d[c * 128:(c + 1) * 128, :])
                wt.append(t)
            wt8 = []
            for j in range(3):
                t = wt_p.tile([128, 2 * 3 * C], F8E4, tag="wt8")
                nc.sync.dma_start(t[:], w8_d[j])
                wt8.append(t[:].rearrange("p (s f) -> p s f", f=3 * C))

            pend_tr = []

            def emit_transposes(qkq_t, b, n0, ns):
                for j in range(12):
                    pt = ps_tr.tile([128, 128], BF16, tag="ps_tr")
                    nc.tensor.transpose(pt[:, :ns], qkq_t[:ns, j * 128:(j + 1) * 128],
                                        ident[:ns, :ns])
                    dst = qT[b][j] if j < 6 else kT[b][j - 6]
                    if j % 3 != 1:
                        nc.vector.tensor_copy(dst[:, n0:n0 + ns], pt[:, :ns])
                    else:
                        nc.scalar.copy(dst[:, n0:n0 + ns], pt[:, :ns])

            for b in (range(BPC) if _PH in ("full", "A") else []):
                xs = []
                for c in range(6):
                    xh = xs_p.tile([128, N], BF16, tag="xh")
                    nc.sync.dma_start(xh[:], xh_d[b, c * 128:(c + 1) * 128, :])
                    xs.append(xh)
                xl8 = []
                for j in range(3):
                    t = xs_p.tile([128, 2 * 1040], F8E4, tag="xl8")
                    nc.sync.dma_start(t[:], xl_d[b, j])
                    xl8.append(t[:].rearrange("p (s f) -> p s f", f=1040))

                for nci, (n0, ns) in enumerate(N_CHUNKS):
                    qkv_sb = qkvsb_p.tile([128, 3 * C], F32, tag="qkvsb")
                    if ns == 1:
                        # n=1024 row: qkv precomputed on host (exact fp32)
                        nc.sync.dma_start(qkv_sb[0:1, :], qt_d[b:b + 1, :])
                    else:
                        pss = []
                        for ti, (o0, osz) in enumerate(QKV_TILES):
                            ps = ps_qkv.tile([128, 512], F32, tag="ps_qkv")
                            for c in range(6):
                                nc.tensor.matmul(
                                    ps[:ns, :osz],
                                    xs[c][:, n0:n0 + ns],
                                    wt[c][:, o0:o0 + osz],
                                    start=(c == 0), stop=False)
                            # lo-pass: fp8 DoubleRow, 1/512 folded into wt8
                            for j in range(3):
                                nc.tensor.matmul(
                                    ps[:ns, :osz],
                                    xl8[j][:, :, n0:n0 + ns],
                                    wt8[j][:, :, o0:o0 + osz],
                                    start=False, stop=(j == 2),
                                    perf_mode=mybir.MatmulPerfMode.DoubleRow)
                            pss.append((ps, o0, osz))
                        for ti, (ps, o0, osz) in enumerate(pss):
                            if ti in (0, 2):
                                nc.vector.tensor_copy(qkv_sb[:ns, o0:o0 + osz], ps[:ns, :osz])
                            else:
                                nc.scalar.copy(qkv_sb[:ns, o0:o0 + osz], ps[:ns, :osz])
                    # l1 over D-segments: [ns, 36] (DVE; gpsimd can't free-reduce)
                    l1 = small_p.tile([128, 36], F32, tag="l1")
                    nc.vector.tensor_reduce(
                        l1[:ns, :], qkv_sb[:ns, :].rearrange("p (s d) -> p s d", d=D),
                        axis=mybir.AxisListType.X, op=mybir.AluOpType.add,
                        apply_absolute_value=True)
                    rho = small_p.tile([128, 36], F32, tag="rho")
                    nc.vector.tensor_scalar(l1[:ns, :], l1[:ns, :], float(S_CONST), None,
                                            op0=mybir.AluOpType.mult)
                    nc.vector.reciprocal(rho[:ns, :], l1[:ns, :])
                    # u*rho + 192 -> bf16 (rounds); 36 segs split DVE/Pool/ACT
                    y192 = y192_p.tile([128, 3 * C], BF16, tag="y192")
                    for s in range(36):
                        sl_in = qkv_sb[:ns, s * D:(s + 1) * D]
                        sl_out = y192[:ns, s * D:(s + 1) * D]
                        r = s % 3
                        if r == 0:
                            nc.vector.tensor_scalar(
                                sl_out, sl_in, rho[:ns, s:s + 1], M192,
                                op0=mybir.AluOpType.mult, op1=mybir.AluOpType.add)
                        elif r == 1:
                            nc.gpsimd.tensor_scalar(
                                sl_out, sl_in, rho[:ns, s:s + 1], M192,
                                op0=mybir.AluOpType.mult, op1=mybir.AluOpType.add)
                        else:
                            nc.scalar.activation(
                                sl_out, sl_in, mybir.ActivationFunctionType.Identity,
                                bias=M192, scale=rho[:ns, s:s + 1])
                    # q ternary via ACT Sign, k via DVE clamp+sub -> qkq
                    qkq = qkq_p.tile([128, 2 * C], BF16, tag="qkq")
                    nc.scalar.activation(qkq[:ns, 0:C], y192[:ns, 0:C],
                                         mybir.ActivationFunctionType.Sign, bias=-M192)
                    nc.vector.tensor_scalar(
                        y192[:ns, C:2 * C], y192[:ns, C:2 * C], 193.0, 191.0,
                        op0=mybir.AluOpType.min, op1=mybir.AluOpType.max)
                    nc.vector.tensor_scalar(
                        qkq[:ns, C:2 * C], y192[:ns, C:2 * C], M192, None,
                        op0=mybir.AluOpType.subtract)
                    # v ternary via DVE clamp + sub -> vq
                    nc.vector.tensor_scalar(
                        y192[:ns, 2 * C:], y192[:ns, 2 * C:], 193.0, 191.0,
                        op0=mybir.AluOpType.min, op1=mybir.AluOpType.max)
                    nc.vector.tensor_scalar(
                        vq[b][:ns, nci * C:(nci + 1) * C], y192[:ns, 2 * C:],
                        M192, None, op0=mybir.AluOpType.subtract)
                    if pend_tr:
                        emit_transposes(*pend_tr.pop())
                    pend_tr.append((qkq, b, n0, ns))
                if pend_tr:
                    emit_transposes(*pend_tr.pop())

        # ================= PHASE B: attention =================
        with tc.tile_pool(name="attnsb", bufs=2) as as_p, \
             tc.tile_pool(name="ytile", bufs=2) as y_p, \
             tc.tile_pool(name="mt", bufs=2) as mt_p, \
             tc.tile_pool(name="acc", bufs=2) as acc_p, \
             tc.tile_pool(name="stat", bufs=4) as st_p, \
             tc.tile_pool(name="outsb", bufs=3) as os_p, \
             tc.tile_pool(name="ps_qk", bufs=2, space="PSUM") as ps_qk, \
             tc.tile_pool(name="ps_av", bufs=3, space="PSUM") as ps_av, \
             tc.tile_pool(name="ps_tl", bufs=1, space="PSUM") as ps_tl:
            def emit_batch_prep(b):
                """colsum(vq), batched m=1024 attn rows for all heads, vtail bcast."""
                # --- colsum of vq, column-oriented: out [128, 6], col j = c-chunk ---
                cvps = ps_tl.tile([128, 16], F32, tag="ps_tl", name=f"cvps_{b}")
                for cc in range(6):
                    for mi in range(9):
                        ms = 128 if mi < 8 else 1
                        nc.tensor.matmul(
                            cvps[:, cc:cc + 1],
                            vq[b][:ms, mi * C + cc * 128:mi * C + (cc + 1) * 128],
                            ones128b[:ms, :],
                            start=(mi == 0), stop=(mi == 8))
                nc.vector.tensor_copy(cvcol[b][:, 0:6], cvps[:, 0:6])
                # --- batched m-tail: attn[1024, n] for all 12 heads ---
                ktd = mt_p.tile([128, 72], BF16, tag="ktd", name=f"ktd_{b}")
                nc.vector.memset(ktd[:], 0.0)
                for h in range(H):
                    j, r0 = h // 2, (h % 2) * 64
                    nc.vector.tensor_copy(ktd[r0:r0 + 64, j * 12 + h:j * 12 + h + 1],
                                          kT[b][j][r0:r0 + 64, 1024:1025])
                mtps = ps_qk.tile([128, 1024], F32, tag="ps_qk", name=f"mtps_{b}")
                mtt = ps_tl.tile([128, 16], F32, tag="ps_tl", name=f"mtt_{b}")
                for j in range(6):
                    for (n0, nt) in NT2:
                        nc.tensor.matmul(
                            mtps[0:12, n0:n0 + nt],
                            ktd[:, j * 12:(j + 1) * 12],
                            qT[b][j][:, n0:n0 + nt],
                            start=(j == 0), stop=(j == 5))
                    # attn[1024,1024] for all heads -> [12, 1]
                    nc.tensor.matmul(
                        mtt[0:12, 0:1],
                        ktd[:, j * 12:(j + 1) * 12],
                        qT[b][j][:, 1024:1025],
                        start=(j == 0), stop=(j == 5))
                mt_sb = mt_p.tile([12, AS], BF16, tag="mt", name=f"mt_{b}")
                nc.scalar.copy(mt_sb[:, 0:1024], mtps[0:12, 0:1024])
                nc.vector.tensor_copy(mt_sb[:, 1024:1025], mtt[0:12, 0:1])
                # per-head |.| partials over the m-tail row: [12, 1]
                mta = mt_p.tile([12, 1], F32, tag="mta", name=f"mta_{b}")
                nc.vector.tensor_reduce(
                    mta[:], mt_sb[:, 0:1025],
                    axis=mybir.AxisListType.X, op=mybir.AluOpType.add,
                    apply_absolute_value=True)
                # transpose to a [1, 12] row so per-head scalars are partition-0
                nc.tensor.matmul(mtt[0:1, 2:14], mta[:], identf[0:12, 0:12],
                                 start=True, stop=True)
                mta_row = mt_p.tile([1, 12], F32, tag="mtarow", name=f"mtarow_{b}")
                nc.vector.tensor_copy(mta_row[:], mtt[0:1, 2:14])
                # v tail row broadcast to partitions 0..11 for per-head AV
                vtb = mt_p.tile([12, C], BF16, tag="vtb", name=f"vtb_{b}")
                nc.gpsimd.partition_broadcast(vtb[:], vq[b][0:1, 8 * C:9 * C])
                return mt_sb, mta_row, vtb

            def emit_qk(b, h, ptail):
                j, r0 = h // 2, (h % 2) * 64
                qkt = []
                for mi, (m0, ms) in enumerate(M_FULL):
                    ps = ps_qk.tile([128, 1024], F32, tag="ps_qk")
                    for (n0, nt) in NT2:
                        nc.tensor.matmul(
                            ps[:ms, n0:n0 + nt],
                            kT[b][j][r0:r0 + 64, m0:m0 + ms],
                            qT[b][j][r0:r0 + 64, n0:n0 + nt],
                            start=True, stop=True)
                    nc.tensor.matmul(
                        ptail[:ms, mi:mi + 1],
                        kT[b][j][r0:r0 + 64, m0:m0 + ms],
                        qT[b][j][r0:r0 + 64, 1024:1025],
                        start=True, stop=True)
                    qkt.append((ps, mi, ms))
                return qkt

            def emit_drain_stats(b, h, attn_sb, yt, qkt, ptail, acc, rho_b, mta_row):
                for (ps, mi, ms) in qkt:
                    dst = attn_sb[:ms, mi * AS:mi * AS + 1024]
                    if mi in (0, 1):
                        nc.vector.tensor_copy(dst, ps[:ms, :1024])
                    else:
                        nc.scalar.copy(dst, ps[:ms, :1024])
                chunks3d = attn_sb[:].rearrange("p (s d) -> p s d", d=AS)
                nc.vector.tensor_copy(
                    chunks3d[:, :, 1024:1025],
                    ptail[:, 0:8].rearrange("p (s d) -> p s d", d=1))

                # stats: DVE chunks 0-2, ACT chunks 3-7 (abs+accum), mtail partial
                nc.vector.tensor_reduce(
                    acc[:, 0:3], chunks3d[:, 0:3, 0:1025],
                    axis=mybir.AxisListType.X, op=mybir.AluOpType.add,
                    apply_absolute_value=True)
                # abs scratch lands in yt (quant overwrites it afterwards)
                yt3d = yt[:].rearrange("p (s d) -> p s d", d=AS)
                nc.scalar.activation(
                    yt3d[:, 3:8, 0:1025], chunks3d[:, 3:8, 0:1025],
                    mybir.ActivationFunctionType.Abs, accum_out=acc[:, 3:4])
                # fold this head's m-tail partial in at partition 0, then
                # all-reduce across partitions on Pool (keeps PE out of the chain)
                nc.vector.memset(acc[:, 4:5], 0.0)
                nc.vector.tensor_copy(acc[0:1, 4:5], mta_row[0:1, h:h + 1])
                nc.vector.tensor_reduce(
                    acc[:, 5:6], acc[:, 0:5],
                    axis=mybir.AxisListType.X, op=mybir.AluOpType.add)
                nc.gpsimd.partition_all_reduce(rho_b[:, 5:6], acc[:, 5:6],
                                               channels=128,
                                               reduce_op=bass_isa.ReduceOp.add)
                nc.vector.tensor_scalar(rho_b[:, 1:2], rho_b[:, 5:6],
                                        1.0 / (N * N), float(C_EPS),
                                        op0=mybir.AluOpType.mult,
                                        op1=mybir.AluOpType.add)
                nc.vector.reciprocal(rho_b[:, 0:1], rho_b[:, 1:2])
                nc.vector.tensor_scalar(rho_b[:, 2:3], rho_b[:, 1:2], float(KAPPA), None,
                                        op0=mybir.AluOpType.mult)
                # -192*colsum(v) bias for this head, partition-aligned to 0..63
                cvs = cvcol[b][(h % 2) * 64:(h % 2) * 64 + 64, h // 2:h // 2 + 1]
                nc.vector.tensor_scalar(rho_b[0:64, 3:4], cvs, -M192, None,
                                        op0=mybir.AluOpType.mult)
                # bias*kt for the ACT osb path
                nc.vector.tensor_scalar(rho_b[0:64, 4:5], rho_b[0:64, 3:4],
                                        rho_b[0:64, 2:3], None,
                                        op0=mybir.AluOpType.mult)

            def emit_quant(attn_sb, yt, ytail, mt_sb, rho_b):
                # y = clamp(bf16(a*rho + 192), 191, 193); groups (0-3), (4-7)
                for g in range(2):
                    sl_in = attn_sb[:, g * 4 * AS:(g + 1) * 4 * AS]
                    sl_out = yt[:, g * 4 * AS:(g + 1) * 4 * AS]
                    nc.vector.tensor_scalar(
                        sl_out, sl_in, rho_b[:, 0:1], M192,
                        op0=mybir.AluOpType.mult, op1=mybir.AluOpType.add)
                    if g == 0:
                        nc.vector.tensor_scalar(
                            sl_out, sl_out, 193.0, 191.0,
                            op0=mybir.AluOpType.min, op1=mybir.AluOpType.max)
                    else:
                        nc.vector.tensor_scalar(
                            yt[:, 4 * AS:6 * AS], yt[:, 4 * AS:6 * AS], 193.0, 191.0,
                            op0=mybir.AluOpType.min, op1=mybir.AluOpType.max)
                        nc.gpsimd.tensor_scalar(
                            yt[:, 6 * AS:8 * AS], yt[:, 6 * AS:8 * AS], 193.0, 191.0,
                            op0=mybir.AluOpType.min, op1=mybir.AluOpType.max)
                # m-tail row for this head
                nc.vector.tensor_scalar(
                    ytail[:], mt_sb[:], rho_b[0:12, 0:1], M192,
                    op0=mybir.AluOpType.mult, op1=mybir.AluOpType.add)
                nc.vector.tensor_scalar(
                    ytail[:], ytail[:], 193.0, 191.0,
                    op0=mybir.AluOpType.min, op1=mybir.AluOpType.max)

            def emit_av(b, h, yt, ytail, vtb, rho_b):
                pso = [ps_av.tile([128, 512], F32, tag="ps_av",
                                  name=f"pso_{b}_{h}_{i}") for i in range(2)]
                pst = ps_av.tile([128, 16], F32, tag="ps_av", name=f"psot_{b}_{h}")
                for mi, (m0, ms) in enumerate(M_FULL):
                    vsl = vq[b][:ms, mi * C + h * D:mi * C + h * D + D]
                    for ti, (n0, nt) in enumerate(NT2):
                        nc.tensor.matmul(
                            pso[ti][:64, :nt], vsl, yt[:ms, mi * AS + n0:mi * AS + n0 + nt],
                            start=(mi == 0), stop=False)
                    nc.tensor.matmul(
                        pst[:64, 0:1], vsl, yt[:ms, mi * AS + 1024:mi * AS + 1025],
                        start=(mi == 0), stop=False)
                # m-tail row contribution: one-hot-masked v-tail, 12-partition contract
                vmask = mt_p.tile([12, D], BF16, tag="vmask")
                nc.vector.tensor_scalar(vmask[:], vtb[0:12, h * D:(h + 1) * D],
                                        identf[0:12, h:h + 1], None,
                                        op0=mybir.AluOpType.mult)
                for ti, (n0, nt) in enumerate(NT2):
                    nc.tensor.matmul(pso[ti][:64, :nt], vmask[:], ytail[0:12, n0:n0 + nt],
                                     start=False, stop=True)
                nc.tensor.matmul(pst[:64, 0:1], vmask[:], ytail[0:12, 1024:1025],
                                 start=False, stop=True)
                return pso, pst

            def emit_out(b, h, pso, pst, rho_b):
                # out = (pso - 192*colsum_v) * kt
                kt_col = rho_b[0:64, 2:3]
                bias_col = rho_b[0:64, 3:4]
                biaskt_col = rho_b[0:64, 4:5]
                for ti, (n0, nt) in enumerate(NT2):
                    osb = os_p.tile([64, 512], F32, tag="outsb")
                    if ti == 0:
                        nc.vector.tensor_scalar(osb[:, :nt], pso[ti][:64, :nt],
                                                bias_col, kt_col,
                                                op0=mybir.AluOpType.add,
                                                op1=mybir.AluOpType.mult)
                    else:
                        nc.scalar.activation(osb[:, :nt], pso[ti][:64, :nt],
                                             mybir.ActivationFunctionType.Identity,
                                             scale=kt_col, bias=biaskt_col)
                    nc.sync.dma_start(y_d[b, h * D:(h + 1) * D, n0:n0 + nt], osb[:, :nt])
                osb = os_p.tile([64, 512], F32, tag="outsb")
                nc.vector.tensor_scalar(osb[:, 0:1], pst[:64, 0:1],
                                        bias_col, kt_col,
                                        op0=mybir.AluOpType.add,
                                        op1=mybir.AluOpType.mult)
                nc.sync.dma_start(y_d[b, h * D:(h + 1) * D, 1024:1025], osb[:, 0:1])

            for b in (range(BPC) if _PH in ("full", "B") else []):
                mt_sb, mta_row, vtb = emit_batch_prep(b)
                prev = None
                for h in range(H):
                    attn_sb = as_p.tile([128, 8 * AS], BF16, tag="attnsb")
                    yt = y_p.tile([128, 8 * AS], BF16, tag="ytile")
                    ytail = mt_p.tile([12, AS], BF16, tag="ytail")
                    acc = acc_p.tile([128, 6], F32, tag="acc")
                    rho_b = st_p.tile([128, 6], F32, tag="rho_b")
                    ptail = ps_tl.tile([128, 16], F32, tag="ps_tl")
                    qkt = emit_qk(b, h, ptail)
                    emit_drain_stats(b, h, attn_sb, yt, qkt, ptail, acc, rho_b, mta_row)
                    if prev is not None:
                        pv = prev
                        pso, pst = emit_av(b, pv["h"], pv["yt"], pv["ytail"], vtb, pv["rho_b"])
                        emit_out(b, pv["h"], pso, pst, pv["rho_b"])
                    emit_quant(attn_sb, yt, ytail, mt_sb, rho_b)
                    prev = {"h": h, "yt": yt, "ytail": ytail, "rho_b": rho_b}
                pv = prev
                pso, pst = emit_av(b, pv["h"], pv["yt"], pv["ytail"], vtb, pv["rho_b"])
                emit_out(b, pv["h"], pso, pst, pv["rho_b"])
    nc.finalize()
    return nc


_NC = None

def _get_nc():
    global _NC
    if _NC is None:
        _NC = build_nc()
    return _NC


def _make_in_maps(x, w_qkv):
    x = np.ascontiguousarray(x, dtype=np.float32)
    w = np.ascontiguousarray(w_qkv, dtype=np.float32)
    s_w = np.float32(np.mean(np.abs(w)) + np.float32(EPS))
    wq_int = np.round(np.clip(w / s_w, -1, 1)).astype(np.float32)  # [3C, C]
    wt = np.ascontiguousarray(wq_int.T).astype(ml_dtypes.bfloat16)  # [C, 3C]
    ident = np.eye(128, dtype=ml_dtypes.bfloat16)
    ones128 = np.ones((128, 1), np.float32)

    xt = x.transpose(0, 2, 1)  # [B, C, N]
    xh = xt.astype(ml_dtypes.bfloat16)
    xl = xt - xh.astype(np.float32)
    # lo residual as e4m3(xl*512); 1/512 folded into wt8 (exact subnormal)
    xl8 = (xl * 512.0).astype(ml_dtypes.float8_e4m3)
    # plane-major [b, j, p, s, n] with planes padded to 1040 (16B-aligned stride)
    xl8p = np.zeros((B, 3, 2, 128, 1040), ml_dtypes.float8_e4m3)
    xl8p[:, :, :, :, :N] = xl8.reshape(B, 3, 2, 128, N)
    xl8r = np.ascontiguousarray(xl8p.transpose(0, 1, 3, 2, 4)).reshape(B, 3, 128, 2 * 1040)
    w8 = (wq_int.T / 512.0).astype(ml_dtypes.float8_e4m3)  # [C, 3C]
    w8r = np.ascontiguousarray(
        w8.reshape(3, 2, 128, 3 * C).transpose(0, 2, 1, 3)
    ).reshape(3, 128, 2 * 3 * C)
    # host-computed qkv for the n=1024 tail row (exact fp32, ternary w folded)
    qkvt = (x[:, 1024, :] @ wq_int.T).astype(np.float32)  # [B, 3C]

    in_maps = []
    for core in range(8):
        sl = slice(core * BPC, (core + 1) * BPC)
        in_maps.append({
            "x_hi": np.ascontiguousarray(xh[sl]),
            "x_lo8": np.ascontiguousarray(xl8r[sl]),
            "wt8": w8r,
            "qkvt": np.ascontiguousarray(qkvt[sl]),
            "wt_bf": wt, "ident": ident, "ones128": ones128,
        })
    return in_maps


def kernel(x, w_qkv):
    in_maps = _make_in_maps(x, w_qkv)
    nc = _get_nc()
    res = run_bass_kernel_spmd(nc, in_maps, core_ids=list(range(8)))
    out = np.empty((B, N, C), np.float32)
    for core in range(8):
        out[core * BPC:(core + 1) * BPC] = res.results[core]["y_sh"].transpose(0, 2, 1)
    return out


# revision 99
# speedup vs baseline: 1.0001x; 1.0001x over previous
"""Trainium2 Bass kernel for ternary-quantized attention (BitNet-style).

Host contract: kernel(x, w_qkv) -> [16,1025,768] fp32.
Shards B=16 over 8 cores (2 batches/core), replicates the ternary weight.

Math (matches fp32 reference to ~0.8% rel err):
  - w ternarized on host to {-1,0,1}; s_w folded out (scale-invariant l1norm).
  - qkv = x_hi @ wt + x_lo8 @ wt8: hi pass in bf16 (1 cyc/row), lo residual
    as e4m3(x_lo*512) via fp8 DoubleRow (0.5 cyc/row) with 1/512 folded into
    wt8 = wq/512 (exact e4m3 subnormal), both accumulating into one PSUM
    group. Host does the split; n=1024 tail row qkv computed on host in fp32.
    Total qkv noise ~2^-14 -> rel err 0.0165 (< 2e-2 gate).
  - q/k/v quantize: u = t / (l1_row * s_const), s_const = 1/64 + 1e-5;
    ternary = sign(bf16(u + 192) - 192)   [bf16 write rounds to int, RNE]
  - attn_int = q_q @ k_q^T (exact ternary bf16 matmuls, fp32 accum)
  - per-(b,h) scale: t = mean|attn_int| + EPS/(scale*s_const^2); rho = 1/t
  - y = clamp(bf16(attn_int*rho + 192), 191, 193) in {191,192,193}
  - out = (y @ v_q - 192*colsum(v_q)) * (scale * s_const^3 * t)
    [-192*colsum folded in as a rank-1 correction matmul into PSUM]
  - the m=1024 attn row is computed for all 12 heads at once via a
    block-diagonal k-tail stationary matrix (per batch, not per head).
"""
import sys, os
sys.path.insert(0, "/opt/trn_rl_repo")
import numpy as np
import ml_dtypes
from contextlib import ExitStack

import concourse.bass as bass
import concourse.tile as tile
from concourse import bacc
from concourse import mybir
from concourse import bass_isa
from concourse.bass_utils import run_bass_kernel_spmd

EPS = 1e-5
B, N, C, H, D = 16, 1025, 768, 12, 64
BPC = B // 8  # batches per core
SCALE = float(D) ** -0.5
S_CONST = np.float32(1.0 / D) + np.float32(EPS)
C_EPS = np.float32(EPS) / (np.float32(SCALE) * S_CONST * S_CONST)
KAPPA = np.float32(SCALE) * S_CONST * S_CONST * S_CONST
M192 = 192.0

F32 = mybir.dt.float32
F32R = mybir.dt.float32r
BF16 = mybir.dt.bfloat16
F8E4 = mybir.dt.float8e4

N_CHUNKS = [(i * 128, 128) for i in range(8)] + [(1024, 1)]
M_FULL = [(i * 128, 128) for i in range(8)]  # m-tail row handled batched
AS = 1026  # attn_sb per-m-chunk column stride (even)
QKV_TILES = [(0, 512), (512, 512), (1024, 512), (1536, 512), (2048, 256)]
NT2 = ((0, 512), (512, 512))


def build_nc():
    nc = bacc.Bacc("TRN2", target_bir_lowering=False, debug=False,
                   enable_asserts=False, num_devices=8)
    for val in (-M192, M192):
        t = nc.alloc_sbuf_tensor(f"const-f32-{val}", [128, 1], F32)
        nc.gpsimd.memset(t.ap(), val)
        nc.const_aps.aps[(F32, val)] = t.ap()
    nc.all_engine_barrier()
    xh_d = nc.dram_tensor("x_hi", [BPC, C, N], BF16, kind="ExternalInput").ap()
    # x_lo as e4m3(x_lo*512), c-dim grouped (3 pairs, 2 subtiles, 128) for DoubleRow
    xl_d = nc.dram_tensor("x_lo8", [BPC, 3, 128, 2 * 1040], F8E4, kind="ExternalInput").ap()
    w8_d = nc.dram_tensor("wt8", [3, 128, 2 * 3 * C], F8E4, kind="ExternalInput").ap()
    wt_d = nc.dram_tensor("wt_bf", [C, 3 * C], BF16, kind="ExternalInput").ap()
    qt_d = nc.dram_tensor("qkvt", [BPC, 3 * C], F32, kind="ExternalInput").ap()
    id_d = nc.dram_tensor("ident", [128, 128], BF16, kind="ExternalInput").ap()
    on_d = nc.dram_tensor("ones128", [128, 1], F32, kind="ExternalInput").ap()
    y_d = nc.dram_tensor("y_sh", [BPC, C, N], F32, kind="ExternalOutput").ap()

    with tile.TileContext(nc) as tc, ExitStack() as ctx:
        const_p = ctx.enter_context(tc.tile_pool(name="consts", bufs=1))
        qt_p = ctx.enter_context(tc.tile_pool(name="qt", bufs=6 * BPC))
        kt_p = ctx.enter_context(tc.tile_pool(name="kt", bufs=6 * BPC))
        vq_p = ctx.enter_context(tc.tile_pool(name="vq", bufs=BPC))
        cv_p = ctx.enter_context(tc.tile_pool(name="cv", bufs=BPC))

        ident = const_p.tile([128, 128], BF16, tag="ident")
        nc.sync.dma_start(ident[:], id_d)
        ones128 = const_p.tile([128, 1], F32, tag="ones")
        nc.sync.dma_start(ones128[:], on_d)
        ones128b = const_p.tile([128, 1], BF16, tag="onesb")
        nc.vector.tensor_copy(ones128b[:], ones128[:])
        identf = const_p.tile([128, 128], F32, tag="identf")
        nc.vector.tensor_copy(identf[:], ident[:])

        qT = [[qt_p.tile([128, N], BF16, tag="qt", name=f"qT_{b}_{j}") for j in range(6)] for b in range(BPC)]
        kT = [[kt_p.tile([128, N], BF16, tag="kt", name=f"kT_{b}_{j}") for j in range(6)] for b in range(BPC)]
        vq = [vq_p.tile([128, 9 * C], BF16, tag="vq", name=f"vq_{b}") for b in range(BPC)]
        # colsum(vq) as columns: [128, 6]; col j = c-chunk j (2 heads stacked)
        cvcol = [cv_p.tile([128, 8], F32, tag="cv", name=f"cv_{b}") for b in range(BPC)]

        _PH = os.environ.get("KERNEL_PHASE", "full")
        # ================= PHASE A: qkv + quantize + transpose =================
        with tc.tile_pool(name="wt", bufs=6) as wt_p, \
             tc.tile_pool(name="xs", bufs=6) as xs_p, \
             tc.tile_pool(name="qkvsb", bufs=2) as qkvsb_p, \
             tc.tile_pool(name="small_a", bufs=4) as small_p, \
             tc.tile_pool(name="y192", bufs=2) as y192_p, \
             tc.tile_pool(name="qkq", bufs=2) as qkq_p, \
             tc.tile_pool(name="ps_qkv", bufs=5, space="PSUM") as ps_qkv, \
             tc.tile_pool(name="ps_tr", bufs=3, space="PSUM") as ps_tr:
            wt = []
            for c in range(6):
                t = wt_p.tile([128, 3 * C], BF16, tag="wt")
                nc.sync.dma_start(t[:], wt_d[c * 128:(c + 1) * 128, :])
                wt.append(t)
            wt8 = []
            for j in range(3):
                t = wt_p.tile([128, 2 * 3 * C], F8E4, tag="wt8")
                nc.sync.dma_start(t[:], w8_d[j])
                wt8.append(t[:].rearrange("p (s f) -> p s f", f=3 * C))

            pend_tr = []

            def emit_transposes(qkq_t, b, n0, ns):
                for j in range(12):
                    pt = ps_tr.tile([128, 128], BF16, tag="ps_tr")
                    nc.tensor.transpose(pt[:, :ns], qkq_t[:ns, j * 128:(j + 1) * 128],
                                        ident[:ns, :ns])
                    dst = qT[b][j] if j < 6 else kT[b][j - 6]
                    if j % 3 != 1:
                        nc.vector.tensor_copy(dst[:, n0:n0 + ns], pt[:, :ns])
                    else:
                        nc.scalar.copy(dst[:, n0:n0 + ns], pt[:, :ns])

            for b in (range(BPC) if _PH in ("full", "A") else []):
                xs = []
                for c in range(6):
                    xh = xs_p.tile([128, N], BF16, tag="xh")
                    nc.sync.dma_start(xh[:], xh_d[b, c * 128:(c + 1) * 128, :])
                    xs.append(xh)
                xl8 = []
                for j in range(3):
                    t = xs_p.tile([128, 2 * 1040], F8E4, tag="xl8")
                    nc.sync.dma_start(t[:], xl_d[b, j])
                    xl8.append(t[:].rearrange("p (s f) -> p s f", f=1040))

                for nci, (n0, ns) in enumerate(N_CHUNKS):
                    qkv_sb = qkvsb_p.tile([128, 3 * C], F32, tag="qkvsb")
                    if ns == 1:
                        # n=1024 row: qkv precomputed on host (exact fp32)
                        nc.sync.dma_start(qkv_sb[0:1, :], qt_d[b:b + 1, :])
                    else:
                        pss = []
                        for ti, (o0, osz) in enumerate(QKV_TILES):
                            ps = ps_qkv.tile([128, 512], F32, tag="ps_qkv")
                            for c in range(6):
                                nc.tensor.matmul(
                                    ps[:ns, :osz],
                                    xs[c][:, n0:n0 + ns],
                                    wt[c][:, o0:o0 + osz],
                                    start=(c == 0), stop=False)
                            # lo-pass: fp8 DoubleRow, 1/512 folded into wt8
                            for j in range(3):
                                nc.tensor.matmul(
                                    ps[:ns, :osz],
                                    xl8[j][:, :, n0:n0 + ns],
                                    wt8[j][:, :, o0:o0 + osz],
                                    start=False, stop=(j == 2),
                                    perf_mode=mybir.MatmulPerfMode.DoubleRow)
                            pss.append((ps, o0, osz))
                        for ti, (ps, o0, osz) in enumerate(pss):
                            if ti in (0, 2):
                                nc.vector.tensor_copy(qkv_sb[:ns, o0:o0 + osz], ps[:ns, :osz])
                            else:
                                nc.scalar.copy(qkv_sb[:ns, o0:o0 + osz], ps[:ns, :osz])
                    # l1 over D-segments: [ns, 36] (DVE; gpsimd can't free-reduce)
                    l1 = small_p.tile([128, 36], F32, tag="l1")
                    nc.vector.tensor_reduce(
                        l1[:ns, :], qkv_sb[:ns, :].rearrange("p (s d) -> p s d", d=D),
                        axis=mybir.AxisListType.X, op=mybir.AluOpType.add,
                        apply_absolute_value=True)
                    rho = small_p.tile([128, 36], F32, tag="rho")
                    nc.vector.tensor_scalar(l1[:ns, :], l1[:ns, :], float(S_CONST), None,
                                            op0=mybir.AluOpType.mult)
                    nc.vector.reciprocal(rho[:ns, :], l1[:ns, :])
                    # u*rho + 192 -> bf16 (rounds); 36 segs split DVE/Pool/ACT
                    y192 = y192_p.tile([128, 3 * C], BF16, tag="y192")
                    for s in range(36):
                        sl_in = qkv_sb[:ns, s * D:(s + 1) * D]
                        sl_out = y192[:ns, s * D:(s + 1) * D]
                        r = s % 3
                        if r == 0:
                            nc.vector.tensor_scalar(
                                sl_out, sl_in, rho[:ns, s:s + 1], M192,
                                op0=mybir.AluOpType.mult, op1=mybir.AluOpType.add)
                        elif r == 1:
                            nc.gpsimd.tensor_scalar(
                                sl_out, sl_in, rho[:ns, s:s + 1], M192,
                                op0=mybir.AluOpType.mult, op1=mybir.AluOpType.add)
                        else:
                            nc.scalar.activation(
                                sl_out, sl_in, mybir.ActivationFunctionType.Identity,
                                bias=M192, scale=rho[:ns, s:s + 1])
                    # q ternary via ACT Sign, k via DVE clamp+sub -> qkq
                    qkq = qkq_p.tile([128, 2 * C], BF16, tag="qkq")
                    nc.scalar.activation(qkq[:ns, 0:C], y192[:ns, 0:C],
                                         mybir.ActivationFunctionType.Sign, bias=-M192)
                    nc.vector.tensor_scalar(
                        y192[:ns, C:2 * C], y192[:ns, C:2 * C], 193.0, 191.0,
                        op0=mybir.AluOpType.min, op1=mybir.AluOpType.max)
                    nc.vector.tensor_scalar(
                        qkq[:ns, C:2 * C], y192[:ns, C:2 * C], M192, None,
                        op0=mybir.AluOpType.subtract)
                    # v ternary via DVE clamp + sub -> vq
                    nc.vector.tensor_scalar(
                        y192[:ns, 2 * C:], y192[:ns, 2 * C:], 193.0, 191.0,
                        op0=mybir.AluOpType.min, op1=mybir.AluOpType.max)
                    nc.vector.tensor_scalar(
                        vq[b][:ns, nci * C:(nci + 1) * C], y192[:ns, 2 * C:],
                        M192, None, op0=mybir.AluOpType.subtract)
                    if pend_tr:
                        emit_transposes(*pend_tr.pop())
                    pend_tr.append((qkq, b, n0, ns))
                if pend_tr:
                    emit_transposes(*pend_tr.pop())

        # ================= PHASE B: attention =================
        with tc.tile_pool(name="attnsb", bufs=2) as as_p, \
             tc.tile_pool(name="ytile", bufs=2) as y_p, \
             tc.tile_pool(name="mt", bufs=2) as mt_p, \
             tc.tile_pool(name="acc", bufs=2) as acc_p, \
             tc.tile_pool(name="stat", bufs=4) as st_p, \
             tc.tile_pool(name="outsb", bufs=6) as os_p, \
             tc.tile_pool(name="ps_qk", bufs=2, space="PSUM") as ps_qk, \
             tc.tile_pool(name="ps_av", bufs=3, space="PSUM") as ps_av, \
             tc.tile_pool(name="ps_tl", bufs=1, space="PSUM") as ps_tl:
            def emit_batch_prep(b):
                """colsum(vq), batched m=1024 attn rows for all heads, vtail bcast."""
                # --- colsum of vq, column-oriented: out [128, 6], col j = c-chunk ---
                cvps = ps_tl.tile([128, 16], F32, tag="ps_tl", name=f"cvps_{b}")
                for cc in range(6):
                    for mi in range(9):
                        ms = 128 if mi < 8 else 1
                        nc.tensor.matmul(
                            cvps[:, cc:cc + 1],
                            vq[b][:ms, mi * C + cc * 128:mi * C + (cc + 1) * 128],
                            ones128b[:ms, :],
                            start=(mi == 0), stop=(mi == 8))
                nc.vector.tensor_copy(cvcol[b][:, 0:6], cvps[:, 0:6])
                # --- batched m-tail: attn[1024, n] for all 12 heads ---
                ktd = mt_p.tile([128, 72], BF16, tag="ktd", name=f"ktd_{b}")
                nc.vector.memset(ktd[:], 0.0)
                for h in range(H):
                    j, r0 = h // 2, (h % 2) * 64
                    nc.vector.tensor_copy(ktd[r0:r0 + 64, j * 12 + h:j * 12 + h + 1],
                                          kT[b][j][r0:r0 + 64, 1024:1025])
                mtps = ps_qk.tile([128, 1024], F32, tag="ps_qk", name=f"mtps_{b}")
                mtt = ps_tl.tile([128, 16], F32, tag="ps_tl", name=f"mtt_{b}")
                for j in range(6):
                    for (n0, nt) in NT2:
                        nc.tensor.matmul(
                            mtps[0:12, n0:n0 + nt],
                            ktd[:, j * 12:(j + 1) * 12],
                            qT[b][j][:, n0:n0 + nt],
                            start=(j == 0), stop=(j == 5))
                    # attn[1024,1024] for all heads -> [12, 1]
                    nc.tensor.matmul(
                        mtt[0:12, 0:1],
                        ktd[:, j * 12:(j + 1) * 12],
                        qT[b][j][:, 1024:1025],
                        start=(j == 0), stop=(j == 5))
                mt_sb = mt_p.tile([12, AS], BF16, tag="mt", name=f"mt_{b}")
                nc.scalar.copy(mt_sb[:, 0:1024], mtps[0:12, 0:1024])
                nc.vector.tensor_copy(mt_sb[:, 1024:1025], mtt[0:12, 0:1])
                # per-head |.| partials over the m-tail row: [12, 1]
                mta = mt_p.tile([12, 1], F32, tag="mta", name=f"mta_{b}")
                nc.vector.tensor_reduce(
                    mta[:], mt_sb[:, 0:1025],
                    axis=mybir.AxisListType.X, op=mybir.AluOpType.add,
                    apply_absolute_value=True)
                # transpose to a [1, 12] row so per-head scalars are partition-0
                nc.tensor.matmul(mtt[0:1, 2:14], mta[:], identf[0:12, 0:12],
                                 start=True, stop=True)
                mta_row = mt_p.tile([1, 12], F32, tag="mtarow", name=f"mtarow_{b}")
                nc.vector.tensor_copy(mta_row[:], mtt[0:1, 2:14])
                # v tail row broadcast to partitions 0..11 for per-head AV
                vtb = mt_p.tile([12, C], BF16, tag="vtb", name=f"vtb_{b}")
                nc.gpsimd.partition_broadcast(vtb[:], vq[b][0:1, 8 * C:9 * C])
                return mt_sb, mta_row, vtb

            def emit_qk(b, h, ptail):
                j, r0 = h // 2, (h % 2) * 64
                qkt = []
                for mi, (m0, ms) in enumerate(M_FULL):
                    ps = ps_qk.tile([128, 1024], F32, tag="ps_qk")
                    for (n0, nt) in NT2:
                        nc.tensor.matmul(
                            ps[:ms, n0:n0 + nt],
                            kT[b][j][r0:r0 + 64, m0:m0 + ms],
                            qT[b][j][r0:r0 + 64, n0:n0 + nt],
                            start=True, stop=True)
                    nc.tensor.matmul(
                        ptail[:ms, mi:mi + 1],
                        kT[b][j][r0:r0 + 64, m0:m0 + ms],
                        qT[b][j][r0:r0 + 64, 1024:1025],
                        start=True, stop=True)
                    qkt.append((ps, mi, ms))
                return qkt

            def emit_drain_stats(b, h, attn_sb, yt, qkt, ptail, acc, rho_b, mta_row):
                for (ps, mi, ms) in qkt:
                    dst = attn_sb[:ms, mi * AS:mi * AS + 1024]
                    if mi in (0, 1):
                        nc.vector.tensor_copy(dst, ps[:ms, :1024])
                    else:
                        nc.scalar.copy(dst, ps[:ms, :1024])
                chunks3d = attn_sb[:].rearrange("p (s d) -> p s d", d=AS)
                nc.vector.tensor_copy(
                    chunks3d[:, :, 1024:1025],
                    ptail[:, 0:8].rearrange("p (s d) -> p s d", d=1))

                # stats: DVE chunks 0-2, ACT chunks 3-7 (abs+accum), mtail partial
                nc.vector.tensor_reduce(
                    acc[:, 0:3], chunks3d[:, 0:3, 0:1025],
                    axis=mybir.AxisListType.X, op=mybir.AluOpType.add,
                    apply_absolute_value=True)
                # abs scratch lands in yt (quant overwrites it afterwards)
                yt3d = yt[:].rearrange("p (s d) -> p s d", d=AS)
                nc.scalar.activation(
                    yt3d[:, 3:8, 0:1025], chunks3d[:, 3:8, 0:1025],
                    mybir.ActivationFunctionType.Abs, accum_out=acc[:, 3:4])
                # fold this head's m-tail partial in at partition 0, then
                # all-reduce across partitions on Pool (keeps PE out of the chain)
                nc.vector.memset(acc[:, 4:5], 0.0)
                nc.vector.tensor_copy(acc[0:1, 4:5], mta_row[0:1, h:h + 1])
                nc.vector.tensor_reduce(
                    acc[:, 5:6], acc[:, 0:5],
                    axis=mybir.AxisListType.X, op=mybir.AluOpType.add)
                nc.gpsimd.partition_all_reduce(rho_b[:, 5:6], acc[:, 5:6],
                                               channels=128,
                                               reduce_op=bass_isa.ReduceOp.add)
                nc.vector.tensor_scalar(rho_b[:, 1:2], rho_b[:, 5:6],
                                        1.0 / (N * N), float(C_EPS),
                                        op0=mybir.AluOpType.mult,
                                        op1=mybir.AluOpType.add)
                nc.vector.reciprocal(rho_b[:, 0:1], rho_b[:, 1:2])
                nc.vector.tensor_scalar(rho_b[:, 2:3], rho_b[:, 1:2], float(KAPPA), None,
                                        op0=mybir.AluOpType.mult)
                # -192*colsum(v) bias for this head, partition-aligned to 0..63
                cvs = cvcol[b][(h % 2) * 64:(h % 2) * 64 + 64, h // 2:h // 2 + 1]
                nc.vector.tensor_scalar(rho_b[0:64, 3:4], cvs, -M192, None,
                                        op0=mybir.AluOpType.mult)
                # bias*kt for the ACT osb path
                nc.vector.tensor_scalar(rho_b[0:64, 4:5], rho_b[0:64, 3:4],
                                        rho_b[0:64, 2:3], None,
                                        op0=mybir.AluOpType.mult)

            def emit_quant(attn_sb, yt, ytail, mt_sb, rho_b):
                # y = clamp(bf16(a*rho + 192), 191, 193); groups (0-3), (4-7)
                for g in range(2):
                    sl_in = attn_sb[:, g * 4 * AS:(g + 1) * 4 * AS]
                    sl_out = yt[:, g * 4 * AS:(g + 1) * 4 * AS]
                    nc.vector.tensor_scalar(
                        sl_out, sl_in, rho_b[:, 0:1], M192,
                        op0=mybir.AluOpType.mult, op1=mybir.AluOpType.add)
                    if g == 0:
                        nc.vector.tensor_scalar(
                            sl_out, sl_out, 193.0, 191.0,
                            op0=mybir.AluOpType.min, op1=mybir.AluOpType.max)
                    else:
                        nc.vector.tensor_scalar(
                            yt[:, 4 * AS:6 * AS], yt[:, 4 * AS:6 * AS], 193.0, 191.0,
                            op0=mybir.AluOpType.min, op1=mybir.AluOpType.max)
                        nc.gpsimd.tensor_scalar(
                            yt[:, 6 * AS:8 * AS], yt[:, 6 * AS:8 * AS], 193.0, 191.0,
                            op0=mybir.AluOpType.min, op1=mybir.AluOpType.max)
                # m-tail row for this head
                nc.vector.tensor_scalar(
                    ytail[:], mt_sb[:], rho_b[0:12, 0:1], M192,
                    op0=mybir.AluOpType.mult, op1=mybir.AluOpType.add)
                nc.vector.tensor_scalar(
                    ytail[:], ytail[:], 193.0, 191.0,
                    op0=mybir.AluOpType.min, op1=mybir.AluOpType.max)

            def emit_av(b, h, yt, ytail, vtb, rho_b):
                pso = [ps_av.tile([128, 512], F32, tag="ps_av",
                                  name=f"pso_{b}_{h}_{i}") for i in range(2)]
                pst = ps_av.tile([128, 16], F32, tag="ps_av", name=f"psot_{b}_{h}")
                for mi, (m0, ms) in enumerate(M_FULL):
                    vsl = vq[b][:ms, mi * C + h * D:mi * C + h * D + D]
                    for ti, (n0, nt) in enumerate(NT2):
                        nc.tensor.matmul(
                            pso[ti][:64, :nt], vsl, yt[:ms, mi * AS + n0:mi * AS + n0 + nt],
                            start=(mi == 0), stop=False)
                    nc.tensor.matmul(
                        pst[:64, 0:1], vsl, yt[:ms, mi * AS + 1024:mi * AS + 1025],
                        start=(mi == 0), stop=False)
                # m-tail row contribution: one-hot-masked v-tail, 12-partition contract
                vmask = mt_p.tile([12, D], BF16, tag="vmask")
                nc.vector.tensor_scalar(vmask[:], vtb[0:12, h * D:(h + 1) * D],
                                        identf[0:12, h:h + 1], None,
                                        op0=mybir.AluOpType.mult)
                for ti, (n0, nt) in enumerate(NT2):
                    nc.tensor.matmul(pso[ti][:64, :nt], vmask[:], ytail[0:12, n0:n0 + nt],
                                     start=False, stop=True)
                nc.tensor.matmul(pst[:64, 0:1], vmask[:], ytail[0:12, 1024:1025],
                                 start=False, stop=True)
                return pso, pst

            def emit_out(b, h, pso, pst, rho_b):
                # out = (pso - 192*colsum_v) * kt
                kt_col = rho_b[0:64, 2:3]
                bias_col = rho_b[0:64, 3:4]
                biaskt_col = rho_b[0:64, 4:5]
                for ti, (n0, nt) in enumerate(NT2):
                    osb = os_p.tile([64, 512], F32, tag="outsb")
                    if ti == 0:
                        nc.vector.tensor_scalar(osb[:, :nt], pso[ti][:64, :nt],
                                                bias_col, kt_col,
                                                op0=mybir.AluOpType.add,
                                                op1=mybir.AluOpType.mult)
                    else:
                        nc.scalar.activation(osb[:, :nt], pso[ti][:64, :nt],
                                             mybir.ActivationFunctionType.Identity,
                                             scale=kt_col, bias=biaskt_col)
                    nc.sync.dma_start(y_d[b, h * D:(h + 1) * D, n0:n0 + nt], osb[:, :nt])
                osb = os_p.tile([64, 512], F32, tag="outsb")
                nc.vector.tensor_scalar(osb[:, 0:1], pst[:64, 0:1],
                                        bias_col, kt_col,
                                        op0=mybir.AluOpType.add,
                                        op1=mybir.AluOpType.mult)
                nc.sync.dma_start(y_d[b, h * D:(h + 1) * D, 1024:1025], osb[:, 0:1])

            for b in (range(BPC) if _PH in ("full", "B") else []):
                mt_sb, mta_row, vtb = emit_batch_prep(b)
                prev = None
                for h in range(H):
                    attn_sb = as_p.tile([128, 8 * AS], BF16, tag="attnsb")
                    yt = y_p.tile([128, 8 * AS], BF16, tag="ytile")
                    ytail = mt_p.tile([12, AS], BF16, tag="ytail")
                    acc = acc_p.tile([128, 6], F32, tag="acc")
                    rho_b = st_p.tile([128, 6], F32, tag="rho_b")
                    ptail = ps_tl.tile([128, 16], F32, tag="ps_tl")
                    qkt = emit_qk(b, h, ptail)
                    emit_drain_stats(b, h, attn_sb, yt, qkt, ptail, acc, rho_b, mta_row)
                    if prev is not None:
                        pv = prev
                        pso, pst = emit_av(b, pv["h"], pv["yt"], pv["ytail"], vtb, pv["rho_b"])
                        emit_out(b, pv["h"], pso, pst, pv["rho_b"])
                    emit_quant(attn_sb, yt, ytail, mt_sb, rho_b)
                    prev = {"h": h, "yt": yt, "ytail": ytail, "rho_b": rho_b}
                pv = prev
                pso, pst = emit_av(b, pv["h"], pv["yt"], pv["ytail"], vtb, pv["rho_b"])
                emit_out(b, pv["h"], pso, pst, pv["rho_b"])
    nc.finalize()
    return nc


_NC = None

def _get_nc():
    global _NC
    if _NC is None:
        _NC = build_nc()
    return _NC


def _make_in_maps(x, w_qkv):
    x = np.ascontiguousarray(x, dtype=np.float32)
    w = np.ascontiguousarray(w_qkv, dtype=np.float32)
    s_w = np.float32(np.mean(np.abs(w)) + np.float32(EPS))
    wq_int = np.round(np.clip(w / s_w, -1, 1)).astype(np.float32)  # [3C, C]
    wt = np.ascontiguousarray(wq_int.T).astype(ml_dtypes.bfloat16)  # [C, 3C]
    ident = np.eye(128, dtype=ml_dtypes.bfloat16)
    ones128 = np.ones((128, 1), np.float32)

    xt = x.transpose(0, 2, 1)  # [B, C, N]
    xh = xt.astype(ml_dtypes.bfloat16)
    xl = xt - xh.astype(np.float32)
    # lo residual as e4m3(xl*512); 1/512 folded into wt8 (exact subnormal)
    xl8 = (xl * 512.0).astype(ml_dtypes.float8_e4m3)
    # plane-major [b, j, p, s, n] with planes padded to 1040 (16B-aligned stride)
    xl8p = np.zeros((B, 3, 2, 128, 1040), ml_dtypes.float8_e4m3)
    xl8p[:, :, :, :, :N] = xl8.reshape(B, 3, 2, 128, N)
    xl8r = np.ascontiguousarray(xl8p.transpose(0, 1, 3, 2, 4)).reshape(B, 3, 128, 2 * 1040)
    w8 = (wq_int.T / 512.0).astype(ml_dtypes.float8_e4m3)  # [C, 3C]
    w8r = np.ascontiguousarray(
        w8.reshape(3, 2, 128, 3 * C).transpose(0, 2, 1, 3)
    ).reshape(3, 128, 2 * 3 * C)
    # host-computed qkv for the n=1024 tail row (exact fp32, ternary w folded)
    qkvt = (x[:, 1024, :] @ wq_int.T).astype(np.float32)  # [B, 3C]

    in_maps = []
    for core in range(8):
        sl = slice(core * BPC, (core + 1) * BPC)
        in_maps.append({
            "x_hi": np.ascontiguousarray(xh[sl]),
            "x_lo8": np.ascontiguousarray(xl8r[sl]),
            "wt8": w8r,
            "qkvt": np.ascontiguousarray(qkvt[sl]),
            "wt_bf": wt, "ident": ident, "ones128": ones128,
        })
    return in_maps


def kernel(x, w_qkv):
    in_maps = _make_in_maps(x, w_qkv)
    nc = _get_nc()
    res = run_bass_kernel_spmd(nc, in_maps, core_ids=list(range(8)))
    out = np.empty((B, N, C), np.float32)
    for core in range(8):
        out[core * BPC:(core + 1) * BPC] = res.results[core]["y_sh"].transpose(0, 2, 1)
    return out
